# revision 8
# baseline (speedup 1.0000x reference)
# Trainium2 Bass kernel for nn_ChannelAttentionBlock:
#   per batch b: F = x[b].reshape(4096, 128)  (raw row-major view)
#                A = F @ F.T            [4096, 4096]
#                P = softmax(A, axis=-1)
#                out[b] = (F.T @ P).reshape(128, 64, 64)
#
# Sharding: data-parallel over batch — B=8 batches, one per NeuronCore.
#
# Per-core algorithm (X := F.T as [128, 4096], n-blocked by 128 rows of A):
#   prologue: DMA F row-blocks (contiguous) -> XT tiles; PE-transpose them
#             into Xr = float32r(F.T); negdiag[n] = -sum_k F[n,k]^2 (softmax
#             shift — any per-row constant is mathematically exact; the
#             diagonal dominates each row of A for this Gram matrix, making
#             exp overflow-safe).
#   per row-block i: A_i = Xr[:,blk_i].T @ Xr (8 matmuls of [128,512]),
#             P~_i = exp(A_i + negdiag_i) via ScalarE with per-partition bias
#             and accum_out giving the row sums; fold 1/s into the O-matmul's
#             stationary operand: Xs_i = XT_i * (1/s_i);
#             O += Xs_i.T @ P~_i: columns 0:2048 accumulate in PSUM across all
#             32 blocks, columns 2048:4096 go through transient PSUM and a
#             VectorE add into an SBUF accumulator (PSUM has only 8 banks).
#
# Matmuls run in float32r (TF32-like, 1 cycle/row at N=512 — 4x faster than
# plain fp32, ~16x more accurate than bf16; measured rel err ~1.5e-4).

import numpy as np

import concourse.bass as bass
import concourse.mybir as mybir
import concourse.tile as tile
from concourse.bass_utils import run_bass_kernel_spmd

N_CORES = 8
D = 128          # feature dim / partition dim
N = 4096         # sequence dim (64*64)
NB = N // 128    # 32 row blocks
F32 = mybir.dt.float32
F32R = mybir.dt.float32r
AX = mybir.AxisListType
ALU = mybir.AluOpType
ACT = mybir.ActivationFunctionType


def _split_waits(nc, max_waits=1):
    """walrus in this toolchain encodes at most 1 semaphore wait per
    instruction; Tile emits several on its tail drain. Move overflow waits
    onto preceding same-engine NoOps (sequencer executes them in order)."""
    n_split = 0
    for f in nc.m.functions:
        for bb in f.blocks:
            new_insts = []
            for inst in bb.instructions:
                si = inst.sync_info
                if si is not None and si.on_wait and len(si.on_wait) > max_waits:
                    waits = list(si.on_wait)
                    chunks = [waits[i:i + max_waits]
                              for i in range(0, len(waits), max_waits)]
                    for chunk in chunks[:-1]:
                        nop = mybir.InstNoOp(
                            name=nc.get_next_instruction_name(), ins=[], outs=[])
                        nop.engine = inst.engine
                        nop.sync_info = mybir.SyncInfo(on_wait=chunk, on_update=[])
                        new_insts.append(nop)
                        n_split += 1
                    inst.sync_info = mybir.SyncInfo(
                        on_wait=chunks[-1],
                        on_update=list(si.on_update) if si.on_update else [])
                new_insts.append(inst)
            bb.instructions = new_insts
    return n_split


def _build_nc():
    nc = bass.Bass("TRN2", target_bir_lowering=False, debug=False)
    x_d = nc.dram_tensor("x", [N, D], F32, kind="ExternalInput").ap()
    id_d = nc.dram_tensor("ident", [D, D], F32, kind="ExternalInput").ap()
    y_d = nc.dram_tensor("y", [D, N], F32, kind="ExternalOutput").ap()

    with tile.TileContext(nc) as tc:
        with tc.tile_pool(name="const", bufs=1) as const, \
             tc.tile_pool(name="ppool", bufs=18) as ppool, \
             tc.tile_pool(name="loop", bufs=8) as loop, \
             tc.tile_pool(name="ores", bufs=1, space="PSUM") as ores_pool, \
             tc.tile_pool(name="trans", bufs=1, space="PSUM") as trans_pool:

            XT = const.tile([D, N], F32, tag="XT")      # XT[:,128i:..] = F[blk_i,:]
            Xr = const.tile([D, N], F32R, tag="Xr")     # rounded F.T
            O_hi = const.tile([D, 3072], F32, tag="Ohi")
            negdiag = const.tile([D, NB], F32, tag="negdiag")
            ident = const.tile([D, D], F32, tag="ident")

            nc.sync.dma_start(ident[:], id_d[:])
            # XT[p, 128i+k] = x_d[128i+p, k]; contiguous 512B bursts per row.
            x_r = x_d.rearrange("(i p) k -> p i k", p=D)
            XT_v = XT[:].rearrange("p (i k) -> p i k", k=D)
            for g in range(8):
                nc.sync.dma_start(XT_v[:, g * 4:(g + 1) * 4, :],
                                  x_r[:, g * 4:(g + 1) * 4, :])

            # Prologue: build Xr = f32r(F.T) via PE transposes of XT blocks,
            # 4 transposes batched per PSUM tile with one evacuation copy,
            # alternating between ScalarE and VectorE (both idle here).
            for g in range(8):
                tp = trans_pool.tile([D, 512], F32, tag="ta", bufs=2)
                for u in range(4):
                    i = 4 * g + u
                    nc.tensor.transpose(tp[:, u * D:(u + 1) * D],
                                        XT[:, i * D:(i + 1) * D], ident[:])
                dst = Xr[:, g * 512:(g + 1) * 512]
                if g % 2 == 0:
                    nc.scalar.copy(dst, tp[:])
                else:
                    nc.vector.tensor_copy(dst, tp[:])

            O_res = ores_pool.tile([D, 1024], F32, tag="ores")

            # Software-pipelined main loop. A-chunks own two dedicated PSUM
            # slots (tag "ta") so the scalar engine's exp stream (the
            # bottleneck) is never gated on O-side work. O columns 1024:4096
            # flow through ONE transient PSUM slot (tag "tot") that
            # accumulates a PAIR of blocks before each VectorE add into the
            # SBUF accumulator, halving the DVE add traffic.
            state = {}  # block -> (pchunks, spart)

            def a_phase(i):
                lhsA = Xr[:, i * D:(i + 1) * D]
                # negdiag[p, i] = -sum_k F[128i+p, k]^2 = -A[n,n] for n=128i+p
                nsq = loop.tile([D, D], F32, tag="nsq")
                nc.vector.scalar_tensor_tensor(
                    nsq[:], XT[:, i * D:(i + 1) * D], -1.0,
                    XT[:, i * D:(i + 1) * D], op0=ALU.mult, op1=ALU.mult)
                nc.vector.tensor_reduce(negdiag[:, i:i + 1], nsq[:],
                                        axis=AX.X, op=ALU.add)
                spart = loop.tile([D, 4], F32, tag="spart")
                pchunks = []
                for c in range(4):
                    At = trans_pool.tile([D, 1024], F32, tag="ta", bufs=2)
                    nc.tensor.matmul(At[:, 0:512], lhsA,
                                     Xr[:, c * 1024:c * 1024 + 512],
                                     start=True, stop=True)
                    nc.tensor.matmul(At[:, 512:1024], lhsA,
                                     Xr[:, c * 1024 + 512:(c + 1) * 1024],
                                     start=True, stop=True)
                    P_c = ppool.tile([D, 1024], F32R, tag="p")
                    nc.scalar.activation(P_c[:], At[:], ACT.Exp,
                                         bias=negdiag[:, i:i + 1],
                                         accum_out=spart[:, c:c + 1])
                    pchunks.append(P_c)
                state[i] = (pchunks, spart)

            def s_chain(i):
                pchunks, spart = state.pop(i)
                s = loop.tile([D, 1], F32, tag="s")
                nc.vector.tensor_reduce(s[:], spart[:], axis=AX.X, op=ALU.add)
                r = loop.tile([D, 1], F32, tag="r")
                nc.vector.reciprocal(r[:], s[:])
                Xs = loop.tile([D, D], F32R, tag="xs")
                nc.gpsimd.tensor_scalar_mul(Xs[:], XT[:, i * D:(i + 1) * D], r[:])
                return pchunks, Xs

            def o_pair(j):
                i0, i1 = 2 * j, 2 * j + 1
                p0, Xs0 = s_chain(i0)
                p1, Xs1 = s_chain(i1)
                # O columns 0:1024 — accumulate in PSUM across all 32 blocks
                for i, Xs, pch in ((i0, Xs0, p0), (i1, Xs1, p1)):
                    for c in range(2):
                        nc.tensor.matmul(
                            O_res[:, c * 512:(c + 1) * 512], Xs[:],
                            pch[0][:, c * 512:(c + 1) * 512],
                            start=(i == 0), stop=(i == NB - 1),
                            skip_group_check=True)
                # O columns 1024:4096 — one transient slot, pair-accumulated
                for h in range(3):
                    Ot = trans_pool.tile([D, 1024], F32, tag="tot", bufs=1)
                    for c in range(2):
                        sl = slice(c * 512, (c + 1) * 512)
                        nc.tensor.matmul(Ot[:, sl], Xs0[:], p0[1 + h][:, sl],
                                         start=True, stop=False,
                                         skip_group_check=True)
                        nc.tensor.matmul(Ot[:, sl], Xs1[:], p1[1 + h][:, sl],
                                         start=False, stop=True,
                                         skip_group_check=True)
                    dst = O_hi[:, h * 1024:(h + 1) * 1024]
                    if j == 0:
                        nc.vector.tensor_copy(dst, Ot[:])
                    else:
                        nc.vector.tensor_add(dst, dst, Ot[:])

            a_phase(0)
            a_phase(1)
            for j in range(NB // 2):
                if 2 * j + 2 < NB:
                    a_phase(2 * j + 2)
                    a_phase(2 * j + 3)
                o_pair(j)

            O_lo = const.tile([D, 1024], F32, tag="olo")
            nc.vector.tensor_copy(O_lo[:], O_res[:])
            nc.sync.dma_start(y_d[:, 0:1024], O_lo[:])
            nc.sync.dma_start(y_d[:, 1024:4096], O_hi[:])

    _split_waits(nc)
    return nc


_NC = None


def _get_nc():
    global _NC
    if _NC is None:
        _NC = _build_nc()
    return _NC


def _in_maps(x):
    ident = np.eye(D, dtype=np.float32)
    return [{"x": np.ascontiguousarray(x[b].reshape(N, D)), "ident": ident}
            for b in range(N_CORES)]


def kernel(x):
    x = np.asarray(x)
    assert x.shape == (N_CORES, D, 64, 64), x.shape
    res = run_bass_kernel_spmd(_get_nc(), _in_maps(x),
                               core_ids=list(range(N_CORES)))
    out = np.stack([res.results[b]["y"] for b in range(N_CORES)])
    return out.reshape(N_CORES, D, 64, 64).astype(np.float32)
